# revision 1
# baseline (speedup 1.0000x reference)
"""2D DWT (db4, circular pad, stride-2) forward on 8 Trainium2 NeuronCores.

Strategy (pure data parallel, 12 images of 512x512 per core):
Both separable filter passes are expressed as banded matmuls on the
TensorEngine, so no transposes are needed anywhere:

  stage 1 (filter along H):  V[w, (hj,a)]   = sum_h  X[h, w] * M[h, (hj,a)]
  stage 2 (filter along W):  out[hj,(wj,b)] = sum_w  V[w, a*256+hj] * M[w, (wj,b)]

M is the 512x512 interleaved filter-bank matrix M[i, 2j+f] = dec[f][(i-2j)%512]
(8 nonzeros per column). Each 128-row chunk of M only has ~67 nonzero j
columns, so each PSUM accumulation streams just the banded column slices
(~536 of 2048 columns per bank) instead of dense 512-wide matmuls.

Precision/speed: fp32 matmuls stream at 4 cycles/row; fp16 streams at 1.
Each product x*m is computed as 3 fp16 matmuls accumulated in fp32 PSUM
(xh*mh + xh*ml + xl*mh with x = xh + xl, m = mh + ml split into fp16
high/low parts) -> full fp32-grade accuracy (~3e-7 rel) at fp16 speed.
X is split on the host (free); V is split on-chip from the PSUM result.
PSUM's per-element has_written bit handles the overlapping column ranges
across K-chunks within one accumulation group.
"""

import sys

sys.path.insert(0, "/opt/trn_rl_repo")

import numpy as np

L = 512
NJ = L // 2  # 256
TAPS = 8
N_CORES = 8
IMGS_PER_CORE = 12  # 32 batch * 3 channels / 8 cores

_compiled = {}


def _build_M(dec: np.ndarray) -> np.ndarray:
    """M[i, 2*j + f] = dec[f][(i - 2j) mod 512]; filters interleaved so each
    128-row chunk's nonzero columns form one contiguous range (plus wrap)."""
    M = np.zeros((L, L), dtype=np.float32)
    i = np.arange(L)[:, None]
    j = np.arange(NJ)[None, :]
    k = (i - 2 * j) % L
    mask = k < TAPS
    for f in range(2):
        M[:, f::2] = np.where(mask, np.asarray(dec[f])[np.minimum(k, TAPS - 1)], 0.0)
    return M


def _col_slices(c: int):
    """Interleaved nonzero column ranges of M rows [128c, 128c+128):
    j in [64c-3, 64c+63] (mod 256) -> interleaved cols [2j, 2j+1]."""
    lo_j, hi_j = 64 * c - 3, 64 * c + 63
    if lo_j < 0:
        return [(0, 2 * (hi_j + 1)), (2 * (lo_j % NJ), 2 * NJ)]
    return [(2 * lo_j, 2 * (hi_j + 1))]


def _group_mms():
    """(chunk, c0, c1) matmul slices for one PSUM accumulation group,
    big slices around the tiny N=6 wrap slice so its LDWEIGHTS exposure
    hides behind long streams (LDW pipelines ~2 deep)."""
    mms = [(c, c0, c1) for c in range(4) for (c0, c1) in _col_slices(c)]
    mms.sort(key=lambda m: -(m[2] - m[1]))
    # [134, 134, 6, 134, 128]
    mms[2], mms[4] = mms[4], mms[2]
    return mms


def _build_nc():
    import concourse.bass as bass  # noqa: F401
    import concourse.tile as tile
    from concourse import bacc, mybir

    f32 = mybir.dt.float32
    f16 = mybir.dt.float16
    nc = bacc.Bacc("TRN2", target_bir_lowering=False, debug=False,
                   num_devices=N_CORES)
    xh_d = nc.dram_tensor("xh", [IMGS_PER_CORE, L, L], f16, kind="ExternalInput")
    xl_d = nc.dram_tensor("xl", [IMGS_PER_CORE, L, L], f16, kind="ExternalInput")
    mh_d = nc.dram_tensor("mh", [L, L], f16, kind="ExternalInput")
    ml_d = nc.dram_tensor("ml", [L, L], f16, kind="ExternalInput")
    o_d = nc.dram_tensor("out", [IMGS_PER_CORE, 4, NJ, NJ], f32,
                         kind="ExternalOutput")

    with tile.TileContext(nc) as tc:
        with (
            tc.tile_pool(name="mpool", bufs=1) as mpool,
            tc.tile_pool(name="xpool", bufs=4) as xpool,
            tc.tile_pool(name="vpool", bufs=3) as vpool,
            tc.tile_pool(name="opool", bufs=6) as opool,
            tc.tile_pool(name="pvpool", bufs=4, space="PSUM") as pvpool,
            tc.tile_pool(name="popool", bufs=4, space="PSUM") as popool,
        ):
            # M hi/lo: 4 h-chunks side by side -> (128, 4*512) fp16 each.
            # mh is issued first so the first group's xh*mh pass can start
            # as soon as mh + xh[0] have landed (ml/xl still in flight).
            mth = mpool.tile([128, 4 * L], f16, tag="mth")
            mtl = mpool.tile([128, 4 * L], f16, tag="mtl")
            nc.sync.dma_start(
                mth[:].rearrange("p (c w) -> p c w", c=4),
                mh_d[:].rearrange("(c p) w -> p c w", p=128),
            )

            for img in range(IMGS_PER_CORE):
                # image hi/lo: 4 h-chunks side by side -> (128, 4*512) fp16
                xht = xpool.tile([128, 4 * L], f16, tag="xht")
                xlt = xpool.tile([128, 4 * L], f16, tag="xlt")
                nc.sync.dma_start(
                    xht[:].rearrange("p (c w) -> p c w", c=4),
                    xh_d[img].rearrange("(c p) w -> p c w", p=128),
                )
                if img == 0:
                    nc.sync.dma_start(
                        mtl[:].rearrange("p (c w) -> p c w", c=4),
                        ml_d[:].rearrange("(c p) w -> p c w", p=128),
                    )
                nc.sync.dma_start(
                    xlt[:].rearrange("p (c w) -> p c w", c=4),
                    xl_d[img].rearrange("(c p) w -> p c w", p=128),
                )

                # stage 1: V[w, (hj,a)], w-chunk wc in v cols [512wc, 512wc+512),
                # de-interleaved: [0:256) = a=0 (lo), [256:512) = a=1 (hi)
                vht = vpool.tile([128, 4 * L], f16, tag="vht")
                vlt = vpool.tile([128, 4 * L], f16, tag="vlt")
                for wc in range(4):
                    pv = pvpool.tile([128, L], f32, tag="pv")
                    mms = _group_mms()
                    terms = [
                        (lt, rt, hc, c0, c1)
                        for (lt, rt) in ((xht, mth), (xht, mtl), (xlt, mth))
                        for (hc, c0, c1) in mms
                    ]
                    for n, (lt, rt, hc, c0, c1) in enumerate(terms):
                        nc.tensor.matmul(
                            pv[:, c0:c1],
                            lt[:, L * hc + 128 * wc : L * hc + 128 * wc + 128],
                            rt[:, L * hc + c0 : L * hc + c1],
                            start=(n == 0),
                            stop=(n == len(terms) - 1),
                        )
                    # de-interleave + fp16 hi/lo split of V (DVE)
                    for f in range(2):
                        dst = slice(L * wc + NJ * f, L * wc + NJ * f + NJ)
                        src = pv[:, f : L : 2]
                        nc.vector.tensor_copy(vht[:, dst], src)
                        nc.vector.tensor_sub(vlt[:, dst], src, vht[:, dst])

                # stage 2: per (a, hjc) one PSUM bank of out[hj, (wj,b)]
                # subband s = a + 2b; ot per hjc: (128, 4*256), free = (s, wj)
                ots = []
                for hjc in range(2):
                    ot = opool.tile([128, 4 * NJ], f32, tag="ot")
                    ots.append(ot)
                    for a in range(2):
                        po = popool.tile([128, L], f32, tag="po")
                        mms = _group_mms()
                        terms = [
                            (lt, rt, wc, c0, c1)
                            for (lt, rt) in ((vht, mth), (vht, mtl), (vlt, mth))
                            for (wc, c0, c1) in mms
                        ]
                        off = NJ * a + 128 * hjc
                        for n, (lt, rt, wc, c0, c1) in enumerate(terms):
                            nc.tensor.matmul(
                                po[:, c0:c1],
                                lt[:, L * wc + off : L * wc + off + 128],
                                rt[:, L * wc + c0 : L * wc + c1],
                                start=(n == 0),
                                stop=(n == len(terms) - 1),
                            )
                        # b=0 (cols 0::2) -> subband a; b=1 (cols 1::2) -> 2+a
                        nc.scalar.copy(ot[:, NJ * a : NJ * a + NJ], po[:, 0:L:2])
                        nc.scalar.copy(
                            ot[:, NJ * (2 + a) : NJ * (2 + a) + NJ], po[:, 1:L:2]
                        )
                for hjc in range(2):
                    nc.sync.dma_start(
                        o_d[img, :, 128 * hjc : 128 * hjc + 128, :].rearrange(
                            "s p w -> p s w"
                        ),
                        ots[hjc][:].rearrange("p (s w) -> p s w", s=4),
                    )

    nc.finalize()
    return nc


def _in_maps(x: np.ndarray, dec: np.ndarray) -> list[dict]:
    M = _build_M(dec)
    mh = M.astype(np.float16)
    ml = (M - mh).astype(np.float16)
    x96 = x.reshape(96, L, L)
    xh = x96.astype(np.float16)
    xl = (x96 - xh).astype(np.float16)
    return [
        {
            "xh": xh[IMGS_PER_CORE * c : IMGS_PER_CORE * (c + 1)],
            "xl": xl[IMGS_PER_CORE * c : IMGS_PER_CORE * (c + 1)],
            "mh": mh,
            "ml": ml,
        }
        for c in range(N_CORES)
    ]


def kernel(x: np.ndarray, dec: np.ndarray) -> np.ndarray:
    from concourse.bass_utils import run_bass_kernel_spmd

    x = np.ascontiguousarray(np.asarray(x, dtype=np.float32))
    dec = np.asarray(dec, dtype=np.float32)
    B, C, H, W = x.shape
    assert (B, C, H, W) == (32, 3, 512, 512) and dec.shape == (2, 8)

    if "nc" not in _compiled:
        _compiled["nc"] = _build_nc()
    nc = _compiled["nc"]

    in_maps = _in_maps(x, dec)
    res = run_bass_kernel_spmd(nc, in_maps, list(range(N_CORES))).results
    out = np.concatenate([r["out"] for r in res], axis=0)  # (96, 4, 256, 256)
    return out.reshape(B, C * 4, H // 2, W // 2)



# revision 4
# speedup vs baseline: 1.6902x; 1.6902x over previous
"""2D DWT (db4, circular pad, stride-2) forward on 8 Trainium2 NeuronCores.

Strategy (pure data parallel, 12 images of 512x512 per core):
Both separable filter passes are banded matmuls on the TensorEngine:

  stage 1 (filter along H):  V[w, (hj,a)]   = sum_h  X[h, w] * M[h, (hj,a)]
  stage 2 (filter along W):  out[hj,(wj,b)] = sum_w  V[w, (hj,a)] * M[w, (wj,b)]

M[i, 2j+f] = dec[f][(i-2j)%512] (8 nonzeros per column). Each 128-row
chunk of M has a 134-wide contiguous band of nonzero columns (wrapping
once), so each PSUM accumulation group streams 536 of 512 columns in 5
banded matmuls instead of dense 512-wide chunks.

The 2e-2 rel-err gate leaves room for a single fp16 pass (measured
~8e-4), so vs the 3-term fp16-split baseline this does 1/3 the matmul
work, half the input DMA (x fp16 once) and half the output DMA (out
fp16, upcast on host). All DRAM<->SBUF transfers are host-pre/post-
shuffled to be fully contiguous, and M is sent band-compacted (137KB).
PSUM->SBUF de-interleave copies are round-robined over DVE/Pool/Act so
no single engine bottlenecks; program order interleaves stage1(img) with
stage2(img-1) so the PE never waits on the V copies.
"""

import sys

sys.path.insert(0, "/opt/trn_rl_repo")

import numpy as np

L = 512
NJ = L // 2  # 256
TAPS = 8
N_CORES = 8
IMGS_PER_CORE = 12  # 32 batch * 3 channels / 8 cores
BW = 134  # nonzero interleaved-column band width per 128-row chunk

_compiled = {}

# Banded matmul slices per accumulation group: (chunk, src0, src1, dst0, dst1)
# src = cols of the compact band tile, dst = cols of the 512-wide PSUM bank.
# Chunk c covers interleaved cols [128c-6, 128c+128) mod 512; c=0 wraps and
# splits in two. Order keeps the tiny 6-wide stream between long ones so its
# weight load hides behind them.
_SLICES = [
    (1, 0, BW, 122, 256),
    (2, 0, BW, 250, 384),
    (0, 0, 6, 506, 512),
    (3, 0, BW, 378, 512),
    (0, 6, BW, 0, 128),
]


def _build_mc(dec: np.ndarray) -> np.ndarray:
    """Compact banded filter matrix: mc[p, c*134 + k] = M[128c+p, (128c-6+k)%512]
    where M[i, 2j+f] = dec[f][(i-2j) % 512] (zero unless (i-2j)%512 < 8)."""
    M = np.zeros((L, L), dtype=np.float32)
    i = np.arange(L)[:, None]
    j = np.arange(NJ)[None, :]
    k = (i - 2 * j) % L
    mask = k < TAPS
    for f in range(2):
        M[:, f::2] = np.where(mask, np.asarray(dec[f])[np.minimum(k, TAPS - 1)], 0.0)
    mc = np.zeros((128, 4 * BW), dtype=np.float16)
    for c in range(4):
        cols = (128 * c - 6 + np.arange(BW)) % L
        mc[:, BW * c : BW * (c + 1)] = M[128 * c : 128 * c + 128, cols]
    return mc


def _build_nc():
    import concourse.bass as bass  # noqa: F401
    import concourse.tile as tile
    from concourse import bacc, mybir

    f32 = mybir.dt.float32
    f16 = mybir.dt.float16
    nc = bacc.Bacc("TRN2", target_bir_lowering=False, debug=False,
                   num_devices=N_CORES)
    x_d = nc.dram_tensor("xc", [IMGS_PER_CORE, 128, 4 * L], f16,
                         kind="ExternalInput")
    mc_d = nc.dram_tensor("mc", [128, 4 * BW], f16, kind="ExternalInput")
    o_d = nc.dram_tensor("out", [IMGS_PER_CORE, 2, 128, 4 * NJ], f16,
                         kind="ExternalOutput")

    with tile.TileContext(nc) as tc:
        with (
            tc.tile_pool(name="mpool", bufs=1) as mpool,
            tc.tile_pool(name="xpool", bufs=4) as xpool,
            tc.tile_pool(name="vpool", bufs=2) as vpool,
            tc.tile_pool(name="opool", bufs=4) as opool,
            tc.tile_pool(name="pvpool", bufs=2, space="PSUM") as pvpool,
            tc.tile_pool(name="popool", bufs=2, space="PSUM") as popool,
        ):
            mct = mpool.tile([128, 4 * BW], f16, tag="mct")
            nc.sync.dma_start(mct[:], mc_d[:])

            # copy engines, round-robined per copy (gpsimd can't read PSUM)
            cp_engines = [
                lambda o, i: nc.vector.tensor_copy(o, i),
                lambda o, i: nc.scalar.copy(o, i),
            ]
            cp_n = [0]

            def copy(o, i):
                cp_engines[cp_n[0] % 2](o, i)
                cp_n[0] += 1

            def group(psum_bank, stationary_of_chunk):
                """One 512-col accumulation group: 5 banded matmuls."""
                for n, (c, s0, s1, d0, d1) in enumerate(_SLICES):
                    nc.tensor.matmul(
                        psum_bank[:, d0:d1],
                        stationary_of_chunk(c),
                        mct[:, BW * c + s0 : BW * c + s1],
                        start=(n == 0),
                        stop=(n == len(_SLICES) - 1),
                    )

            vts = [None, None]  # vt of img, img-1

            def stage1(img, xt):
                vt = vpool.tile([128, 4 * L], f16, tag="vt")
                for pair in range(2):
                    pv = pvpool.tile([128, 2 * L], f32, tag="pv")
                    for wi in range(2):
                        wc = 2 * pair + wi
                        group(
                            pv[:, L * wi : L * wi + L],
                            lambda c: xt[:, L * c + 128 * wc : L * c + 128 * wc + 128],
                        )
                    # de-interleave V: dst (wi, a, j) <- src[p, 512wi + 2j + a]
                    copy(
                        vt[:, 2 * L * pair : 2 * L * (pair + 1)].rearrange(
                            "p (w a j) -> p w a j", w=2, a=2
                        ),
                        pv[:].rearrange("p (w j a) -> p w a j", w=2, j=NJ, a=2),
                    )
                return vt

            def stage2(img, vt):
                for hjc in range(2):
                    po = popool.tile([128, 2 * L], f32, tag="po")
                    ot = opool.tile([128, 4 * NJ], f16, tag="ot")
                    for a in range(2):
                        off = NJ * a + 128 * hjc
                        group(
                            po[:, L * a : L * a + L],
                            lambda c: vt[:, L * c + off : L * c + off + 128],
                        )
                    # subbands: dst (b, a, wj) <- src[p, 512a + 2wj + b]
                    copy(
                        ot[:].rearrange("p (b a w) -> p b a w", b=2, a=2),
                        po[:].rearrange("p (a w b) -> p b a w", a=2, b=2),
                    )
                    nc.sync.dma_start(o_d[img, hjc], ot[:])

            # software pipeline: stage1(img) then stage2(img-1), so stage2's
            # dependence on the V copies never stalls the PE.
            for img in range(IMGS_PER_CORE + 1):
                if img < IMGS_PER_CORE:
                    xt = xpool.tile([128, 4 * L], f16, tag="xt")
                    nc.sync.dma_start(xt[:], x_d[img])
                    vts[0] = stage1(img, xt)
                if img > 0:
                    stage2(img - 1, vts[1])
                vts[1] = vts[0]

    nc.finalize()
    return nc


def _in_maps(x: np.ndarray, dec: np.ndarray) -> list[dict]:
    mc = _build_mc(dec)
    # host pre-shuffle so each image is one contiguous 512KB DMA:
    # xc[i, p, (c, w)] = x[i, 128c + p, w]
    xc = np.ascontiguousarray(
        x.reshape(96, 4, 128, L).astype(np.float16).transpose(0, 2, 1, 3)
    ).reshape(96, 128, 4 * L)
    return [
        {"xc": xc[IMGS_PER_CORE * c : IMGS_PER_CORE * (c + 1)], "mc": mc}
        for c in range(N_CORES)
    ]


def kernel(x: np.ndarray, dec: np.ndarray) -> np.ndarray:
    from concourse.bass_utils import run_bass_kernel_spmd

    x = np.ascontiguousarray(np.asarray(x, dtype=np.float32))
    dec = np.asarray(dec, dtype=np.float32)
    B, C, H, W = x.shape
    assert (B, C, H, W) == (32, 3, 512, 512) and dec.shape == (2, 8)

    if "nc" not in _compiled:
        _compiled["nc"] = _build_nc()
    nc = _compiled["nc"]

    in_maps = _in_maps(x, dec)
    res = run_bass_kernel_spmd(nc, in_maps, list(range(N_CORES))).results
    o = np.concatenate([r["out"] for r in res], axis=0)  # (96, 2, 128, 1024) f16
    # o[i, hjc, p, (s, wj)] -> out[i, s, 128*hjc + p, wj]
    o = o.reshape(96, 2, 128, 4, NJ).transpose(0, 3, 1, 2, 4)
    return np.ascontiguousarray(o, dtype=np.float32).reshape(B, C * 4, H // 2, W // 2)
